# revision 2
# baseline (speedup 1.0000x reference)
"""LIF (leaky integrate-and-fire) spiking recurrence on 8 Trainium2 cores.

Full input x: [T*bs, C, H, W] = [256, 128, 32, 32] f32 with T=8, bs=32.
Recurrence over T only, elementwise elsewhere:
    u_t = TAU * u_{t-1} * (1 - (u_{t-1} > VTH)) + x_t ;  o_t = (u_t > VTH)

Sharding: fully data-parallel over batch (bs=32 -> 4 per core), no collectives.
Each core views its per-timestep slab as [128 partitions, 4096] f32.

The HBM write traffic is the lever: the spikes are 1 bit each, so instead of
storing o as f32 (16 MiB/core) the idle PE packs 16 spike bits into one f32
word (weights 2^i over groups of 16 partitions, exact in bf16 x bf16 -> f32
PSUM since all partial sums are integers < 2^16). Stores drop 16x to 1 MiB
per core and the kernel becomes input-bandwidth-bound (~16.8 MiB in).

Engine split per timestep (columns of the 4096-wide slab):
  D [0:2560):  DVE p-stt: p = (u<=VTH)*u ; DVE u-stt: u' = (p*TAU)+x
               ACT pack source: s = sign(VTH-u); o = relu(-s)  (exact 0/1)
  G [2560:4096) (2 chains of 768):
               DVE m' = (u<=VTH)*TAU -> bf16 (tensor_scalar, 2x mode)
               GPS p' = u*m' (= TAU*p, exact: m' in {0,0.5})
               GPS u' = p' + x
  PE: 8 matmuls pack the bf16 pack-source (o for chunks 0-4, m' for 5-7)
      into PSUM quadrants; ACT copies [128,1024] PSUM->SBUF; SP stores.
Chunks 0-4 decode as o bits directly; chunks 5-7 hold TAU*(1-o) so the host
doubles and complements. Everything is bitwise exact vs the f32 reference.
"""

import numpy as np

import concourse.tile as tile
from concourse import bacc, mybir
from concourse.bass_utils import run_bass_kernel_spmd

T = 8
BS = 32
C = 128
HW = 32 * 32
NCORES = 8
BSH = BS // NCORES          # 4 batch elements per core
P = 128                     # SBUF partitions
FREE = BSH * C * HW // P    # 4096 f32 per partition per timestep
VTH = 1.0
TAU = 0.5
F32 = mybir.dt.float32
BF16 = mybir.dt.bfloat16

WD = 2560                   # D columns: DVE recurrence + ACT o-pack
G1 = (2560, 3328)           # G chains: DVE m' + GPSIMD mult/add
G2 = (3328, 4096)
CHUNK = 512                 # pack matmul moving width
NCHUNK = FREE // CHUNK      # 8 chunks; 0-4 o-type, 5-7 m'-type

_nc_cache = None


def _build():
    nc = bacc.Bacc("TRN2", target_bir_lowering=False, debug=False, num_devices=NCORES)
    x_d = nc.dram_tensor("x", [T, P, FREE], F32, kind="ExternalInput").ap()
    w_d = nc.dram_tensor("w", [P, 8], BF16, kind="ExternalInput").ap()
    pk_d = nc.dram_tensor("pk", [T, 32, 1024], F32, kind="ExternalOutput").ap()

    AL = mybir.AluOpType

    with tile.TileContext(nc) as tc:
        with (
            tc.tile_pool(name="xa", bufs=1) as xa,
            tc.tile_pool(name="wp", bufs=1) as wp,
            tc.tile_pool(name="pp", bufs=2) as pp,
            tc.tile_pool(name="bp", bufs=2) as bp,
            tc.tile_pool(name="sp", bufs=1) as spl,
            tc.tile_pool(name="kp", bufs=2) as kp,
            tc.tile_pool(name="ps", bufs=2, space="PSUM") as ps,
        ):
            # Whole 16 MiB per-core input resident in SBUF (128 KiB/partition);
            # u is computed in place over it.
            xt = xa.tile([P, T * FREE], F32)
            wt = wp.tile([P, 8], BF16)
            nc.sync.dma_start(out=wt, in_=w_d)
            xv = x_d.rearrange("t p f -> p t f")  # [128, T, FREE] HBM view

            # Ramped load sizes (units of 2048 cols): small first so compute
            # starts early, large later so few DMAs cover the rest.
            CHF = 2048
            load_ranges = [(0, 1), (1, 2), (2, 4), (4, 6), (6, 8), (8, 12), (12, 16)]
            for a, b in load_ranges:
                t0, f0 = divmod(a * CHF, FREE)
                t1, f1 = divmod(b * CHF, FREE)
                if f0 == 0 and f1 == 0:
                    src = xv[:, t0:t1, :]
                else:
                    src = xv[:, t0, f0:f1 if f1 else FREE]
                nc.sync.dma_start(out=xt[:, a * CHF:b * CHF], in_=src)

            p_prev = None
            for t in range(T):
                xs = xt[:, t * FREE:(t + 1) * FREE]  # holds u_t in place

                # --- u update (t>0) ---
                if t > 0:
                    # D: u = (p*TAU) + x
                    nc.vector.scalar_tensor_tensor(
                        xs[:, :WD], p_prev[:, :WD], TAU, xs[:, :WD],
                        op0=AL.mult, op1=AL.add,
                    )
                    # G: u = p' + x  (p' already includes TAU)
                    for a, b in (G1, G2):
                        nc.gpsimd.tensor_tensor(
                            xs[:, a:b], p_prev[:, a:b], xs[:, a:b], AL.add,
                        )

                # --- pack source ---
                bt = bp.tile([P, FREE], BF16, name="bt", tag="bt")
                st = spl.tile([P, WD], BF16, name="st", tag="st")
                # D: o = relu(-sign(VTH - u))  (exact {0,1})
                nc.scalar.activation(
                    st, xs[:, :WD], mybir.ActivationFunctionType.Sign,
                    bias=VTH, scale=-1.0,
                )
                nc.scalar.activation(
                    bt[:, :WD], st, mybir.ActivationFunctionType.Relu, scale=-1.0,
                )
                # G: m' = (u <= VTH) * TAU  (exact {0, 0.5}, 2x DVE mode)
                for a, b in (G1, G2):
                    nc.vector.tensor_scalar(
                        bt[:, a:b], xs[:, a:b], VTH, TAU, AL.is_le, AL.mult,
                    )

                # --- p for next step (t < T-1) ---
                if t < T - 1:
                    pn = pp.tile([P, FREE], F32, name="pn", tag="pn")
                    # D: p = (u <= VTH) * u
                    nc.vector.scalar_tensor_tensor(
                        pn[:, :WD], xs[:, :WD], VTH, xs[:, :WD],
                        op0=AL.is_le, op1=AL.mult,
                    )
                    # G: p' = u * m' = TAU * p
                    for a, b in (G1, G2):
                        nc.gpsimd.tensor_tensor(
                            pn[:, a:b], xs[:, a:b], bt[:, a:b], AL.mult,
                        )
                else:
                    pn = None

                # --- pack: 8 matmuls into PSUM quadrants ---
                psum = ps.tile([P, 1024], F32, name="psum", tag="psum")
                for c in range(NCHUNK):
                    pb = 32 * (c % 4)
                    fo = 512 * (c // 4)
                    nc.tensor.matmul(
                        psum[pb:pb + 8, fo:fo + 512],
                        wt,
                        bt[:, CHUNK * c:CHUNK * (c + 1)],
                        start=True, stop=True,
                        tile_position=(0, pb),
                    )
                pkt = kp.tile([P, 1024], F32, name="pkt", tag="pkt")
                nc.scalar.copy(pkt, psum)
                for g in range(4):
                    nc.sync.dma_start(
                        out=pk_d[t, 8 * g:8 * (g + 1), :],
                        in_=pkt[32 * g:32 * g + 8, :],
                    )

                p_prev = pn

    nc.compile()
    return nc


def _get_nc():
    global _nc_cache
    if _nc_cache is None:
        _nc_cache = _build()
    return _nc_cache


def _pack_weights():
    import ml_dtypes
    w = np.zeros((P, 8), dtype=np.float32)
    for p in range(P):
        w[p, p // 16] = float(2 ** (p % 16))
    return w.astype(ml_dtypes.bfloat16)


def _decode(pk):
    """pk: [T, 32, 1024] f32 -> o bits [T, 128, 4096] f32.

    Chunk c of timestep t lives at rows 8*(c%4)+j, cols 512*(c//4)+f with
    value sum_i 2^i * b[16j+i, 512c+f]; b = o for c<5, b = TAU*(1-o) for c>=5.
    """
    v = pk.reshape(T, 4, 8, 2, 512)           # [t, g, j, half, f]
    v = v.transpose(0, 3, 1, 2, 4)            # [t, half, g, j, f]
    v = v.reshape(T, 8, 8, 512)               # [t, c, j, f] with c = 4*half+g
    v = v.copy()
    v[:, 5:] *= 2.0                           # m'-type: 2*TAU*sum = sum of masks
    vi = v.astype(np.uint32).astype(np.uint16)
    bits = np.unpackbits(
        vi.view(np.uint8).reshape(T, 8, 8, 512, 2),
        axis=-1, bitorder="little",
    )                                          # [t, c, j, f*? ...]
    bits = bits.reshape(T, 8, 8, 512, 16)      # [t, c, j, f, i]
    bits[:, 5:] = 1 - bits[:, 5:]              # m'-type: o = NOT mask
    # partition = 16j + i, free = 512c + f
    o = bits.transpose(0, 2, 4, 1, 3)          # [t, j, i, c, f]
    return np.ascontiguousarray(o.reshape(T, P, FREE)).astype(np.float32)


def _run(x: np.ndarray, **spmd_kwargs):
    nc = _get_nc()
    xr = np.ascontiguousarray(np.asarray(x, dtype=np.float32)).reshape(T, BS, C, HW)
    wb = _pack_weights()
    in_maps = [
        {
            "x": np.ascontiguousarray(xr[:, k * BSH:(k + 1) * BSH]).reshape(T, P, FREE),
            "w": wb,
        }
        for k in range(NCORES)
    ]
    res = run_bass_kernel_spmd(nc, in_maps, core_ids=list(range(NCORES)), **spmd_kwargs)
    out = np.empty((T, BS, C, HW), dtype=np.float32)
    for k in range(NCORES):
        o = _decode(res.results[k]["pk"])
        out[:, k * BSH:(k + 1) * BSH] = o.reshape(T, BSH, C, HW)
    return out.reshape(T * BS, C, 32, 32), res


def kernel(x: np.ndarray) -> np.ndarray:
    out, _ = _run(x)
    return out


# revision 3
# speedup vs baseline: 1.0735x; 1.0735x over previous
"""LIF (leaky integrate-and-fire) spiking recurrence on 8 Trainium2 cores.

Full input x: [T*bs, C, H, W] = [256, 128, 32, 32] f32 with T=8, bs=32.
Recurrence over T only, elementwise elsewhere:
    u_t = TAU * u_{t-1} * (1 - (u_{t-1} > VTH)) + x_t ;  o_t = (u_t > VTH)

Sharding: fully data-parallel over batch (bs=32 -> 4 per core), no
collectives. Each core sees a [128, 4096] f32 slab per timestep.

Three tricks get the kernel to the input-bandwidth roofline:

1. Scaled state: track v_t = 2^t * u_t with host-prescaled inputs
   x'_t = 2^t * x_t. Since TAU = 0.5, the recurrence loses its multiply:
       v_{t+1} = v_t * (v_t <= 2^t) + x'_{t+1} ;  o_t = (v_t > 2^t)
   Power-of-two scaling commutes with IEEE f32 ops, so this is bitwise
   exact vs the reference.

2. Accumulating loads: the "+ x'" runs inside the DMA engines via SWDGE
   accum_op=add while the x bytes stream from HBM anyway. The only
   per-timestep compute left is one DVE stt (the masked reset) and the
   spike compare.

3. PE bit-packing: spikes are 1 bit; the idle tensor engine packs 16
   spike bits into one f32 word (weights 2^i over 16-partition groups,
   integer-exact in bf16 x bf16 -> f32 PSUM). HBM writes drop 16x to
   1 MiB/core, so total traffic is ~17.8 MiB/core vs 33.5 baseline.

Spike compare is split DVE (tensor_scalar is_gt, 2x mode) / ACT
(sign+relu, exact) to balance engines. GPSIMD only generates DMA
descriptors - its elementwise path contends with DVE for SBUF ports.
"""

import numpy as np

import concourse.tile as tile
from concourse import bacc, mybir
from concourse.bass_utils import run_bass_kernel_spmd

T = 8
BS = 32
C = 128
HW = 32 * 32
NCORES = 8
BSH = BS // NCORES          # 4 batch elements per core
P = 128                     # SBUF partitions
FREE = BSH * C * HW // P    # 4096 f32 per partition per timestep
VTH = 1.0
TAU = 0.5
F32 = mybir.dt.float32
BF16 = mybir.dt.bfloat16

WA = 2048                   # ACT o-pair columns [WA:FREE); DVE is_gt [0:WA)
NCH = 4                     # recurrence chunk chains (accum pipelining)
CW = FREE // NCH
CHUNK = 512                 # pack matmul moving width

_nc_cache = None


def _build():
    nc = bacc.Bacc("TRN2", target_bir_lowering=False, debug=False, num_devices=NCORES)
    x_d = nc.dram_tensor("x", [T, P, FREE], F32, kind="ExternalInput").ap()
    w_d = nc.dram_tensor("w", [P, 8], BF16, kind="ExternalInput").ap()
    pk_d = nc.dram_tensor("pk", [T, 32, 1024], F32, kind="ExternalOutput").ap()

    AL = mybir.AluOpType

    with tile.TileContext(nc) as tc:
        with (
            tc.tile_pool(name="vp", bufs=3) as vp,
            tc.tile_pool(name="wp", bufs=1) as wp,
            tc.tile_pool(name="bp", bufs=2) as bp,
            tc.tile_pool(name="snp", bufs=2) as snp,
            tc.tile_pool(name="kp", bufs=2) as kp,
            tc.tile_pool(name="ps", bufs=2, space="PSUM") as ps,
        ):
            wt = wp.tile([P, 8], BF16)
            nc.sync.dma_start(out=wt, in_=w_d)

            # v_0 = x'_0, loaded in chunks so chains start early
            vt = vp.tile([P, FREE], F32, name="v0", tag="v")
            for c in range(NCH):
                sl = slice(c * CW, (c + 1) * CW)
                nc.sync.dma_start(out=vt[:, sl], in_=x_d[0][:, sl])

            for t in range(T):
                VT = float(2 ** t)

                # --- spike bits o_t = (v_t > 2^t), bf16 {0,1} ---
                ot = bp.tile([P, FREE], BF16, name="ot", tag="ot")
                nc.vector.tensor_scalar(ot[:, :WA], vt[:, :WA], VT, None, AL.is_gt)
                st = snp.tile([P, FREE - WA], BF16, name="st", tag="st")
                # s = sign(1 - v/2^t)  (exact: v/2^t is a power-of-2 scale)
                nc.scalar.activation(
                    st, vt[:, WA:], mybir.ActivationFunctionType.Sign,
                    bias=1.0, scale=-(2.0 ** -t),
                )
                nc.scalar.activation(
                    ot[:, WA:], st, mybir.ActivationFunctionType.Relu, scale=-1.0,
                )

                # --- next state: v' = v*(v<=2^t) then DMA-accumulate x' ---
                if t < T - 1:
                    vn = vp.tile([P, FREE], F32, name="vn", tag="v")
                    for c in range(NCH):
                        sl = slice(c * CW, (c + 1) * CW)
                        nc.vector.scalar_tensor_tensor(
                            vn[:, sl], vt[:, sl], VT, vt[:, sl],
                            op0=AL.is_le, op1=AL.mult,
                        )
                        nc.gpsimd.dma_start(
                            out=vn[:, sl], in_=x_d[t + 1][:, sl], accum_op=AL.add,
                        )
                else:
                    vn = None

                # --- pack 16 spike bits per f32 word on the PE ---
                psum = ps.tile([P, 1024], F32, name="psum", tag="psum")
                for c in range(FREE // CHUNK):
                    pb = 32 * (c % 4)
                    fo = 512 * (c // 4)
                    nc.tensor.matmul(
                        psum[pb:pb + 8, fo:fo + 512],
                        wt,
                        ot[:, CHUNK * c:CHUNK * (c + 1)],
                        start=True, stop=True,
                        tile_position=(0, pb),
                    )
                pkt = kp.tile([P, 1024], F32, name="pkt", tag="pkt")
                nc.scalar.copy(pkt, psum)
                for g in range(4):
                    nc.sync.dma_start(
                        out=pk_d[t, 8 * g:8 * (g + 1), :],
                        in_=pkt[32 * g:32 * g + 8, :],
                    )

                vt = vn

    nc.compile()
    return nc


def _get_nc():
    global _nc_cache
    if _nc_cache is None:
        _nc_cache = _build()
    return _nc_cache


def _pack_weights():
    import ml_dtypes
    w = np.zeros((P, 8), dtype=np.float32)
    for p in range(P):
        w[p, p // 16] = float(2 ** (p % 16))
    return w.astype(ml_dtypes.bfloat16)


def _decode(pk):
    """pk: [T, 32, 1024] f32 -> o bits [T, 128, 4096] f32.

    Chunk c of timestep t lives at rows 8*(c%4)+j, cols 512*(c//4)+f with
    value sum_i 2^i * o[16j+i, 512c+f].
    """
    v = pk.reshape(T, 4, 8, 2, 512)           # [t, g, j, half, f]
    v = v.transpose(0, 3, 1, 2, 4)            # [t, half, g, j, f]
    v = np.ascontiguousarray(v).reshape(T, 8, 8, 512)  # [t, c, j, f], c=4*half+g
    vi = v.astype(np.uint32).astype(np.uint16)
    bits = np.unpackbits(
        vi.view(np.uint8).reshape(T, 8, 8, 512, 2),
        axis=-1, bitorder="little",
    ).reshape(T, 8, 8, 512, 16)                # [t, c, j, f, i]
    o = bits.transpose(0, 2, 4, 1, 3)          # [t, j, i, c, f]
    return np.ascontiguousarray(o.reshape(T, P, FREE)).astype(np.float32)


def _run(x: np.ndarray, **spmd_kwargs):
    nc = _get_nc()
    xr = np.ascontiguousarray(np.asarray(x, dtype=np.float32)).reshape(T, BS, C, HW)
    scale = (2.0 ** np.arange(T, dtype=np.float32)).reshape(T, 1, 1, 1)
    wb = _pack_weights()
    in_maps = []
    for k in range(NCORES):
        xs = xr[:, k * BSH:(k + 1) * BSH].reshape(T, P, FREE) * scale.reshape(T, 1, 1)
        in_maps.append({"x": np.ascontiguousarray(xs), "w": wb})
    res = run_bass_kernel_spmd(nc, in_maps, core_ids=list(range(NCORES)), **spmd_kwargs)
    out = np.empty((T, BS, C, HW), dtype=np.float32)
    for k in range(NCORES):
        o = _decode(res.results[k]["pk"])
        out[:, k * BSH:(k + 1) * BSH] = o.reshape(T, BSH, C, HW)
    return out.reshape(T * BS, C, 32, 32), res


def kernel(x: np.ndarray) -> np.ndarray:
    out, _ = _run(x)
    return out
